# Initial kernel scaffold
#
"""Trainium2 Bass kernel for nn_DOPAMINEm (DOPAMINE unrolled reconstruction).

Sharding: 8 cores = 2 batches x 4 row-slabs of 48 rows. Each core runs the
300-step pointwise Jr gradient loop on its own 9216 pixels (pixel-partition
layout [128, 72]), then 10 denoiser stages (3x3 convs as 9 shifted
block-diagonal matmuls over a flat padded image buffer), with 5-row halo
exchange between stages via AllGather within each batch's 4-core group.
"""
import math
import os

import numpy as np

import concourse.bacc as bacc
import concourse.bass as bass
import concourse.tile as tile
from concourse import mybir
from concourse.bass_utils import run_bass_kernel_spmd

F32 = mybir.dt.float32
ALU = mybir.AluOpType
ACTF = mybir.ActivationFunctionType

M0_MAX, P2_MAX = 3.0, 10.0
NS, NITER, NMID, FCH = 10, 300, 3, 64
B, H, W, E = 2, 192, 192, 10
N_CORES = 8
SLABS = 4
SH = H // SLABS          # 48
HALO = 5
RW = W + 2               # padded row width 194
NROW = 2 * HALO + SH + 2  # 60 buffer rows (guard + halo + own + halo + guard)
GUARD = 195              # physical guard elems on each side of the flat span
SPAN = NROW * RW         # 11640 logical flat span
XBUF = SPAN + 2 * GUARD  # 12030 physical cols of image-layout buffers
CHUNK0 = RW              # first output flat index (row 1)
NPIX_F = (NROW - 2) * RW  # 11252 flat outputs (rows 1..58)
NCHUNK = math.ceil(NPIX_F / 512)  # 22
NG = 72                  # pixel-layout free groups
EINV = 1.0 / E

_CACHE = {}


# ------------------------- host-side layout helpers -------------------------

def to_pix(a48):
    """[48,192,...] -> [128,72,...] pixel layout."""
    rest = a48.shape[2:]
    return (a48.reshape(16, 3, 8, 24, *rest)
            .transpose(0, 2, 1, 3, *range(4, 4 + len(rest)))
            .reshape(128, 72, *rest))


def from_pix(apix):
    """[128,72,...] -> [48,192,...]."""
    rest = apix.shape[2:]
    return (apix.reshape(16, 8, 3, 24, *rest)
            .transpose(0, 2, 1, 3, *range(4, 4 + len(rest)))
            .reshape(48, 192, *rest))


def pack_weights(ins, ns):
    """Block-diagonal lhsT packing of the two branch denoisers."""
    Wm_in, Wp_in = ins["Wm_in"], ins["Wp_in"]
    Wm_mid, Wp_mid = ins["Wm_mid"], ins["Wp_mid"]
    Wm_out, Wp_out = ins["Wm_out"], ins["Wp_out"]
    w18 = np.zeros((ns, 18, 128), np.float32)
    wmid = np.zeros((ns, NMID, 9, 128, 128), np.float32)
    wout = np.zeros((ns, 9, 128, 2), np.float32)
    for i in range(ns):
        for ky in range(3):
            for kx in range(3):
                t = ky * 3 + kx
                w18[i, 2 * t + 0, 0:64] = Wm_in[i, ky, kx, 0]
                w18[i, 2 * t + 1, 64:128] = Wp_in[i, ky, kx, 0]
                for l in range(NMID):
                    wmid[i, l, t, 0:64, 0:64] = Wm_mid[i, l, ky, kx]
                    wmid[i, l, t, 64:128, 64:128] = Wp_mid[i, l, ky, kx]
                wout[i, t, 0:64, 0] = Wm_out[i, ky, kx, :, 0]
                wout[i, t, 64:128, 1] = Wp_out[i, ky, kx, :, 0]
    bin_ = np.concatenate([ins["bm_in"][:ns], ins["bp_in"][:ns]], axis=1)  # [ns,128]
    bmid = np.concatenate([ins["bm_mid"][:ns], ins["bp_mid"][:ns]], axis=2)  # [ns,3,128]
    bout = np.stack([np.repeat(ins["bm_out"][:ns, 0], 1),
                     np.repeat(ins["bp_out"][:ns, 0], 1)], axis=1)  # [ns,2]
    return w18, wmid, wout, bin_.astype(np.float32), bmid.astype(np.float32), \
        bout.astype(np.float32)


# ------------------------------ device program ------------------------------

def emit_jr(nc, pools, mode, xpix, bpix, tneg, scr):
    """Emit one Jr evaluation on the [128, 720] pixel tiles.

    mode='loop': also applies the x <- rc(x - 2*jr) update in place.
    mode='eval': leaves E*g_m0 in scr.red[:,0,:] and E*g_p2 in scr.gep.
    """
    arg, sr, uz, red, gep = scr
    xm = xpix[:, 0:NG]
    xp = xpix[:, NG:2 * NG]
    xm_b = xm[:, :, None].to_broadcast([128, NG, E])
    xp_b = xp[:, :, None].to_broadcast([128, NG, E])
    tneg_b = tneg[:, None, :].to_broadcast([128, NG, E])
    arg3 = arg.rearrange("p (g k) -> p g k", k=E)
    sr3 = sr.rearrange("p (g k) -> p g k", k=E)
    # arg = -t * p2
    nc.gpsimd.tensor_tensor(out=arg3, in0=tneg_b, in1=xp_b, op=ALU.mult)
    # e = exp(arg)  (in place)
    nc.scalar.activation(out=arg, in_=arg, func=ACTF.Exp)
    # s = e * m0
    nc.gpsimd.tensor_tensor(out=sr3, in0=arg3, in1=xm_b, op=ALU.mult)
    # r = s - b  (in place)
    nc.vector.tensor_tensor(out=sr, in0=sr, in1=bpix, op=ALU.subtract)
    # u = e * r
    nc.vector.tensor_tensor(out=uz[:, 0], in0=arg, in1=sr, op=ALU.mult)
    # z = u * (-t)
    nc.gpsimd.tensor_tensor(out=uz.rearrange("p c (g k) -> p c g k", k=E)[:, 1],
                            in0=uz.rearrange("p c (g k) -> p c g k", k=E)[:, 0],
                            in1=tneg_b, op=ALU.mult)
    # red[:,0]=sum_k u (=E*g0) ; red[:,1]=sum_k z (=E*g1/m0)
    nc.vector.tensor_reduce(out=red, in_=uz.rearrange("p c (g k) -> p c g k", k=E),
                            axis=mybir.AxisListType.X, op=ALU.add)
    # gep = m0 * red1  (=E*g1)
    nc.gpsimd.tensor_tensor(out=gep, in0=xm, in1=red[:, 1], op=ALU.mult)
    if mode == "loop":
        # m0' = clamp(m0 - (2/E) * red0)
        nc.vector.scalar_tensor_tensor(out=xm, in0=red[:, 0], scalar=-2.0 * EINV,
                                       in1=xm, op0=ALU.mult, op1=ALU.add)
        nc.vector.tensor_scalar(out=xm, in0=xm, scalar1=0.0, scalar2=M0_MAX,
                                op0=ALU.max, op1=ALU.min)
        nc.vector.scalar_tensor_tensor(out=xp, in0=gep, scalar=-2.0 * EINV,
                                       in1=xp, op0=ALU.mult, op1=ALU.add)
        nc.vector.tensor_scalar(out=xp, in0=xp, scalar1=0.0, scalar2=P2_MAX,
                                op0=ALU.max, op1=ALU.min)


def build_nc(niter, ns):
    nc = bacc.Bacc("TRN2", target_bir_lowering=False, debug=False,
                   num_devices=N_CORES)
    # ---- I/O ----
    bpix_d = nc.declare_dram_parameter("bpix", [128, NG * E], F32, isOutput=False)
    tneg_d = nc.declare_dram_parameter("tneg", [128, E], F32, isOutput=False)
    w18_d = nc.declare_dram_parameter("w18", [ns, 18, 128], F32, isOutput=False)
    wmid_d = nc.declare_dram_parameter("wmid", [ns, NMID, 9, 128, 128], F32,
                                       isOutput=False)
    wout_d = nc.declare_dram_parameter("wout", [ns, 9, 128, 2], F32, isOutput=False)
    bin_d = nc.declare_dram_parameter("bin", [ns, 128], F32, isOutput=False)
    bmid_d = nc.declare_dram_parameter("bmid", [ns, NMID, 128], F32, isOutput=False)
    bout_d = nc.declare_dram_parameter("bout", [ns, 2], F32, isOutput=False)
    mask_d = nc.declare_dram_parameter("masks", [2, HALO * RW], F32, isOutput=False)
    sc_d = nc.declare_dram_parameter("scal", [ns, 2], F32, isOutput=False)  # [-mu, lm]
    nbr_d = nc.declare_dram_parameter("nbr", [1, 2], mybir.dt.uint32, isOutput=False)
    out_d = nc.declare_dram_parameter("out", [2 * ns + 1, 128, 2 * NG], F32,
                                      isOutput=True)
    # collective bounce buffers
    cc_in = nc.dram_tensor("cc_in", [2, 2 * HALO, RW], F32)
    cc_out = nc.dram_tensor("cc_out", [SLABS, 2, 2 * HALO, RW], F32,
                            addr_space="Shared")

    with tile.TileContext(nc) as tc:
        with (tc.tile_pool(name="big", bufs=1) as big,
              tc.tile_pool(name="wpool", bufs=2) as wpool,
              tc.tile_pool(name="scratch", bufs=1) as scratch,
              tc.tile_pool(name="psum", bufs=8, space="PSUM") as psum):
            # ---- persistent tiles ----
            bpix = big.tile([128, NG * E], F32)
            tneg = big.tile([128, E], F32)
            xpix = big.tile([128, 2 * NG], F32)
            x18 = big.tile([18, XBUF], F32)
            hA = big.tile([128, XBUF], F32)
            hB = big.tile([128, XBUF], F32)
            masks = big.tile([2, HALO * RW], F32)
            nbr = big.tile([1, 2], mybir.dt.uint32)
            arg = scratch.tile([128, NG * E], F32)
            sr = scratch.tile([128, NG * E], F32)
            uz = scratch.tile([128, 2, NG * E], F32)
            red = scratch.tile([128, 2, NG], F32)
            gep = scratch.tile([128, NG], F32)
            scr = (arg, sr, uz, red, gep)
            q = scratch.tile([128, 2 * NG], F32)
            dxc = scratch.tile([128, 2 * NG], F32)
            cres = scratch.tile([128, 2 * NG], F32)

            nc.sync.dma_start(out=bpix[:], in_=bpix_d[:])
            nc.sync.dma_start(out=tneg[:], in_=tneg_d[:])
            nc.sync.dma_start(out=masks[:], in_=mask_d[:])
            nc.sync.dma_start(out=nbr[:], in_=nbr_d[:])
            nc.vector.memset(x18[:], 0.0)
            nc.vector.memset(hA[:], 0.0)
            nc.vector.memset(hB[:], 0.0)

            # ---- init x ----
            bp3 = bpix.rearrange("p (g k) -> p g k", k=E)
            nc.vector.tensor_reduce(out=xpix[:, 0:NG], in_=bp3,
                                    axis=mybir.AxisListType.X, op=ALU.max)
            nc.vector.memset(xpix[:, NG:2 * NG], 1.0)

            # ---- Jr loop ----
            UNROLL = 10
            assert niter % UNROLL == 0
            with tc.For_i(0, niter // UNROLL, 1):
                for _ in range(UNROLL):
                    emit_jr(nc, None, "loop", xpix, bpix, tneg, scr)

            # img-layout helpers -------------------------------------------
            def img_ap(t, ch, base, dims):
                """AP into image buffer row `ch` at logical offset `base`
                with free dims [(step, count), ...]."""
                return bass.AP(
                    tensor=t.tensor,
                    offset=t.offset + ch * t.tensor.shape[1] + GUARD + base,
                    ap=[[0, 1]] + [list(d) for d in dims],
                )

            XIMG = 8  # x18 rows 8,9 double as the center-tap image buffer

            def reshape_pix_to_img(src_pix, dst18):
                # own pixel (r,c): r=(a,rm), c=(d,cm); dst f=(6+3a+rm)*194+1+24d+cm
                for ch in range(2):
                    src = src_pix[:, ch * NG:(ch + 1) * NG]
                    dst = bass.AP(
                        tensor=dst18.tensor,
                        offset=dst18.offset + (XIMG + ch) * dst18.tensor.shape[1]
                        + GUARD + (HALO + 1) * RW + 1,
                        ap=[[0, 1], [3 * RW, 16], [24, 8], [RW, 3], [1, 24]],
                    )
                    nc.sync.dma_start(out=dst, in_=src.rearrange(
                        "p (rm cm) -> p rm cm", cm=24))

            def reshape_img_to_pix(src18, dst_pix):
                for ch in range(2):
                    src = bass.AP(
                        tensor=src18.tensor,
                        offset=src18.offset + (XIMG + ch) * src18.tensor.shape[1]
                        + GUARD + (HALO + 1) * RW + 1,
                        ap=[[0, 1], [3 * RW, 16], [24, 8], [RW, 3], [1, 24]],
                    )
                    nc.sync.dma_start(
                        out=dst_pix[:, ch * NG:(ch + 1) * NG].rearrange(
                            "p (rm cm) -> p rm cm", cm=24),
                        in_=src)

            # registers for neighbor ranks (prev, next within 4-core group)
            prev_r = nc.sync.alloc_register("prev_r")
            next_r = nc.sync.alloc_register("next_r")
            nc.sync.reg_load(prev_r, nbr[0:1, 0:1])
            nc.sync.reg_load(next_r, nbr[0:1, 1:2])

            def halo_exchange(stage):
                # own boundary rows -> cc_in
                top = img_ap(x18, XIMG, (HALO + 1) * RW, [[RW * 0 + 1, 0]])
                # [2, 5*RW] contiguous from row HALO+1 and row HALO+1+SH-5
                src_top = bass.AP(
                    tensor=x18.tensor,
                    offset=x18.offset + XIMG * x18.tensor.shape[1] + GUARD
                    + (HALO + 1) * RW,
                    ap=[[x18.tensor.shape[1], 2], [1, HALO * RW]])
                src_bot = bass.AP(
                    tensor=x18.tensor,
                    offset=x18.offset + XIMG * x18.tensor.shape[1] + GUARD
                    + (1 + SH) * RW,
                    ap=[[x18.tensor.shape[1], 2], [1, HALO * RW]])
                nc.sync.dma_start(out=cc_in[:, 0:HALO, :], in_=src_top)
                nc.sync.dma_start(out=cc_in[:, HALO:2 * HALO, :], in_=src_bot)
                nc.gpsimd.collective_compute(
                    "AllGather", ALU.bypass,
                    replica_groups=[[0, 1, 2, 3], [4, 5, 6, 7]],
                    ins=[cc_in[:]], outs=[cc_out[:]],
                )
                # neighbor rows -> own halo rows of the image buffer
                dst_top = bass.AP(
                    tensor=x18.tensor,
                    offset=x18.offset + XIMG * x18.tensor.shape[1] + GUARD + RW,
                    ap=[[x18.tensor.shape[1], 2], [1, HALO * RW]])
                dst_bot = bass.AP(
                    tensor=x18.tensor,
                    offset=x18.offset + XIMG * x18.tensor.shape[1] + GUARD
                    + (1 + HALO + SH) * RW,
                    ap=[[x18.tensor.shape[1], 2], [1, HALO * RW]])
                nc.sync.dma_start(
                    out=dst_top, in_=cc_out[bass.ds(prev_r, 1), :, HALO:2 * HALO, :])
                nc.sync.dma_start(
                    out=dst_bot, in_=cc_out[bass.ds(next_r, 1), :, 0:HALO, :])
                # mask the halo strips (zero at image boundary, 1 interior)
                for ch in range(2):
                    st = bass.AP(tensor=x18.tensor,
                                 offset=x18.offset + (XIMG + ch) * x18.tensor.shape[1]
                                 + GUARD + RW,
                                 ap=[[0, 1], [1, HALO * RW]])
                    sb = bass.AP(tensor=x18.tensor,
                                 offset=x18.offset + (XIMG + ch) * x18.tensor.shape[1]
                                 + GUARD + (1 + HALO + SH) * RW,
                                 ap=[[0, 1], [1, HALO * RW]])
                    nc.vector.tensor_tensor(out=st, in0=st, in1=masks[0:1, :],
                                            op=ALU.mult)
                    nc.vector.tensor_tensor(out=sb, in0=sb, in1=masks[1:2, :],
                                            op=ALU.mult)

            def build_taps():
                # copy center rows into the other 8 tap rows, shifted
                for ky in range(3):
                    for kx in range(3):
                        t = ky * 3 + kx
                        if t == 4:
                            continue
                        d = (ky - 1) * RW + (kx - 1)
                        src = bass.AP(
                            tensor=x18.tensor,
                            offset=x18.offset + XIMG * x18.tensor.shape[1]
                            + GUARD + d,
                            ap=[[x18.tensor.shape[1], 2], [1, SPAN]])
                        dst = bass.AP(
                            tensor=x18.tensor,
                            offset=x18.offset + 2 * t * x18.tensor.shape[1] + GUARD,
                            ap=[[x18.tensor.shape[1], 2], [1, SPAN]])
                        nc.sync.dma_start(out=dst, in_=src)

            def zero_hpads(h):
                # pad cols 0 and 193 of rows 1..58
                ap = bass.AP(tensor=h.tensor, offset=h.offset + GUARD + RW,
                             ap=[[h.tensor.shape[1], 128], [RW, NROW - 2],
                                 [RW - 1, 2]])
                nc.vector.memset(ap, 0.0)

            def mask_h(h):
                for (base, m) in ((RW, 0), ((1 + HALO + SH) * RW, 1)):
                    ap = bass.AP(tensor=h.tensor, offset=h.offset + GUARD + base,
                                 ap=[[h.tensor.shape[1], 128], [1, HALO * RW]])
                    nc.vector.tensor_tensor(
                        out=ap, in0=ap,
                        in1=masks[m:m + 1, :].to_broadcast([128, HALO * RW]),
                        op=ALU.mult)

            def conv_layer(rhs_tile, rhs_nrow, lhsT, ntap, bias_ap, relu, out_h,
                           out_np, evict_alt):
                """One conv layer: ntap matmuls per chunk accumulating in PSUM,
                then bias(+relu) eviction into out_h rows [0:out_np]."""
                for c in range(NCHUNK):
                    base = CHUNK0 + 512 * c
                    n = min(512, CHUNK0 + NPIX_F - base)
                    ps = psum.tile([128, 512], F32, tag="ps")
                    for t in range(ntap):
                        if ntap == 1:
                            rhs = bass.AP(
                                tensor=rhs_tile.tensor,
                                offset=rhs_tile.offset + GUARD + base,
                                ap=[[rhs_tile.tensor.shape[1], rhs_nrow], [1, n]])
                            nc.tensor.matmul(ps[:out_np, :n], lhsT, rhs,
                                             start=True, stop=True)
                        else:
                            d = ((t // 3) - 1) * RW + (t % 3) - 1
                            rhs = bass.AP(
                                tensor=rhs_tile.tensor,
                                offset=rhs_tile.offset + GUARD + base + d,
                                ap=[[rhs_tile.tensor.shape[1], rhs_nrow], [1, n]])
                            nc.tensor.matmul(ps[:out_np, :n], lhsT[t], rhs,
                                             start=(t == 0), stop=(t == ntap - 1))
                    dst = bass.AP(tensor=out_h.tensor,
                                  offset=out_h.offset + GUARD + base,
                                  ap=[[out_h.tensor.shape[1], out_np], [1, n]])
                    eng = nc.vector if (c % 2 == 0 or not evict_alt) else nc.scalar
                    if relu:
                        if eng is nc.vector:
                            nc.vector.tensor_scalar(
                                out=dst, in0=ps[:out_np, :n], scalar1=bias_ap,
                                scalar2=0.0, op0=ALU.add, op1=ALU.max)
                        else:
                            nc.scalar.activation(out=dst, in_=ps[:out_np, :n],
                                                 func=ACTF.Relu, bias=bias_ap)
                    else:
                        nc.vector.tensor_scalar(
                            out=dst, in0=ps[:out_np, :n], scalar1=bias_ap,
                            scalar2=0.0, op0=ALU.add, op1=ALU.bypass)

            # ---------------- stages ----------------
            for i in range(ns):
                w18s = wpool.tile([18, 128], F32, tag="w18")
                wmids = wpool.tile([128, NMID, 9, 128], F32, tag="wmid")
                wouts = wpool.tile([128, 9 * 2], F32, tag="wout")
                bins = wpool.tile([128, 1], F32, tag="bin")
                bmids = wpool.tile([128, NMID], F32, tag="bmid")
                bouts = wpool.tile([2, 1], F32, tag="bout")
                scs = wpool.tile([128, 2], F32, tag="scs")
                nc.sync.dma_start(out=w18s[:], in_=w18_d[i])
                nc.sync.dma_start(out=wmids[:],
                                  in_=wmid_d[i].rearrange("l t kin m -> kin l t m"))
                nc.sync.dma_start(out=wouts[:],
                                  in_=wout_d[i].rearrange("t kin m -> kin (t m)"))
                nc.sync.dma_start(out=bins[:], in_=bin_d[i, :, None])
                nc.sync.dma_start(out=bmids[:], in_=bmid_d[i].rearrange("l k -> k l"))
                nc.sync.dma_start(out=bouts[:], in_=bout_d[i, :, None])
                nc.sync.dma_start(out=scs[:],
                                  in_=sc_d[i, None, :].to_broadcast([128, 2]))

                reshape_pix_to_img(xpix, x18)
                halo_exchange(i)
                build_taps()
                # conv1: K=18 single matmul per chunk
                conv_layer(x18, 18, w18s[:], 1, bins[:], True, hA, 128, True)
                zero_hpads(hA); mask_h(hA)
                hs = [hA, hB]
                for l in range(NMID):
                    hin, hout = hs[l % 2], hs[(l + 1) % 2]
                    conv_layer(hin, 128,
                               [wmids[:, l, t, :] for t in range(9)], 9,
                               bmids[:, l:l + 1], True, hout, 128, True)
                    zero_hpads(hout); mask_h(hout)
                hlast = hs[NMID % 2]
                # conv5 -> write into x18 rows XIMG (overwrite image; it is
                # no longer needed) -- use hA/hB other buffer? use dedicated:
                conv_layer(hlast, 128,
                           [wouts[:, 2 * t:2 * t + 2] for t in range(9)], 9,
                           bouts[:], False, x18, 2, False)
                # conv result (rows 0,1 of x18) -> pixel layout
                reshape_img_to_pix(x18, cres)
                # Dx = rc(x - cres)
                nc.vector.tensor_tensor(out=dxc, in0=xpix, in1=cres, op=ALU.subtract)
                nc.vector.tensor_scalar(out=dxc[:, 0:NG], in0=dxc[:, 0:NG],
                                        scalar1=0.0, scalar2=M0_MAX,
                                        op0=ALU.max, op1=ALU.min)
                nc.vector.tensor_scalar(out=dxc[:, NG:2 * NG], in0=dxc[:, NG:2 * NG],
                                        scalar1=0.0, scalar2=P2_MAX,
                                        op0=ALU.max, op1=ALU.min)
                nc.sync.dma_start(out=out_d[2 * i], in_=dxc[:])
                # jr eval
                emit_jr(nc, None, "eval", xpix, bpix, tneg, scr)
                # q = x - Dx ; w1 = q*lm ; upd = gE/E + w1 ; x' = rc(x - mu*upd)
                nc.vector.tensor_tensor(out=q, in0=xpix, in1=dxc, op=ALU.subtract)
                nc.vector.tensor_scalar(out=q, in0=q, scalar1=scs[:, 1:2],
                                        scalar2=1.0, op0=ALU.mult, op1=ALU.mult)
                nc.vector.scalar_tensor_tensor(out=q[:, 0:NG], in0=red[:, 0],
                                               scalar=EINV, in1=q[:, 0:NG],
                                               op0=ALU.mult, op1=ALU.add)
                nc.vector.scalar_tensor_tensor(out=q[:, NG:2 * NG], in0=gep,
                                               scalar=EINV, in1=q[:, NG:2 * NG],
                                               op0=ALU.mult, op1=ALU.add)
                # x' = x + negmu*q
                nc.vector.tensor_scalar(out=q, in0=q, scalar1=scs[:, 0:1],
                                        scalar2=1.0, op0=ALU.mult, op1=ALU.mult)
                nc.vector.tensor_tensor(out=xpix, in0=xpix, in1=q, op=ALU.add)
                nc.vector.tensor_scalar(out=xpix[:, 0:NG], in0=xpix[:, 0:NG],
                                        scalar1=0.0, scalar2=M0_MAX,
                                        op0=ALU.max, op1=ALU.min)
                nc.vector.tensor_scalar(out=xpix[:, NG:2 * NG],
                                        in0=xpix[:, NG:2 * NG],
                                        scalar1=0.0, scalar2=P2_MAX,
                                        op0=ALU.max, op1=ALU.min)
                nc.sync.dma_start(out=out_d[2 * i + 1], in_=xpix[:])
            nc.sync.dma_start(out=out_d[2 * ns], in_=xpix[:])
    nc.compile()
    return nc


# ------------------------------- host wrapper -------------------------------

def _prep_inputs(ins, niter, ns):
    b = ins["b"].astype(np.float32)
    tes = ins["tes"].astype(np.float32)
    w18, wmid, wout, bin_, bmid, bout = pack_weights(ins, ns)
    sc = np.stack([-ins["mu"][:ns], ins["lm"][:ns]], axis=1).astype(np.float32)
    in_maps = []
    for core in range(N_CORES):
        bi, s = divmod(core, SLABS)
        bslab = b[bi, s * SH:(s + 1) * SH]          # [48,192,10]
        bpix = to_pix(bslab).reshape(128, NG * E)
        tneg = np.tile(-tes[bi][None, :], (128, 1)).astype(np.float32)
        masks = np.ones((2, HALO * RW), np.float32)
        if s == 0:
            masks[0] = 0.0
        if s == SLABS - 1:
            masks[1] = 0.0
        nbr = np.array([[max(0, s - 1), min(SLABS - 1, s + 1)]], np.uint32)
        in_maps.append({
            "bpix": np.ascontiguousarray(bpix),
            "tneg": tneg,
            "w18": w18[:ns], "wmid": wmid[:ns], "wout": wout[:ns],
            "bin": bin_[:ns], "bmid": bmid[:ns], "bout": bout[:ns],
            "masks": masks, "scal": sc, "nbr": nbr,
        })
    return in_maps


def _run(ins, niter, ns, trace=False):
    key = (niter, ns)
    if key not in _CACHE:
        _CACHE[key] = build_nc(niter, ns)
    nc = _CACHE[key]
    in_maps = _prep_inputs(ins, niter, ns)
    r = run_bass_kernel_spmd(nc, in_maps, core_ids=list(range(N_CORES)),
                             trace=trace)
    nout = 2 * ns + 1
    full = np.zeros((nout, B, H, W, 2), np.float32)
    for core in range(N_CORES):
        bi, s = divmod(core, SLABS)
        o = r.results[core]["out"].reshape(nout, 128, 2, NG)
        o = np.moveaxis(o, 2, 3)                      # [n,128,NG,2]
        full[:, bi, s * SH:(s + 1) * SH] = np.stack(
            [from_pix(o[j]) for j in range(nout)])
    return full, r


def kernel(**inputs):
    full, _ = _run(inputs, NITER, NS)
    return full


# revision 14
# speedup vs baseline: 5.8375x; 5.8375x over previous
"""Trainium2 Bass kernel for nn_DOPAMINEm (DOPAMINE unrolled reconstruction).

Sharding: 8 cores = 2 batches x 4 row-slabs of 48 rows. Each core runs the
300-step pointwise Jr gradient loop on its own 9216 pixels (pixel-partition
layout [128, 72]), then 10 denoiser stages (3x3 convs as 9 shifted
block-diagonal matmuls over a flat padded image buffer), with 5-row halo
exchange between stages via AllGather within each batch's 4-core group.
"""
import math
import os

import numpy as np

import concourse.bacc as bacc
import concourse.bass as bass
import concourse.tile as tile
from concourse import mybir
from concourse.bass_utils import run_bass_kernel_spmd

F32 = mybir.dt.float32
ALU = mybir.AluOpType
ACTF = mybir.ActivationFunctionType

M0_MAX, P2_MAX = 3.0, 10.0
NS, NITER, NMID, FCH = 10, 300, 3, 64
B, H, W, E = 2, 192, 192, 10
N_CORES = 8
SLABS = 4
SH = H // SLABS          # 48
HALO = 5
RW = W + 2               # padded row width 194
NROW = 2 * HALO + SH + 2  # 60 buffer rows (guard + halo + own + halo + guard)
GUARD = 195              # physical guard elems on each side of the flat span
SPAN = NROW * RW         # 11640 logical flat span
XBUF = SPAN + 2 * GUARD  # 12030 physical cols of image-layout buffers
CHUNK0 = RW              # first output flat index (row 1)
NPIX_F = (NROW - 2) * RW  # 11252 flat outputs (rows 1..58)
NCHUNK = math.ceil(NPIX_F / 512)  # 22
NG = 72                  # pixel-layout free groups
EINV = 1.0 / E

_CACHE = {}


# ------------------------- host-side layout helpers -------------------------

def to_pix(a48):
    """[48,192,...] -> [128,72,...] pixel layout."""
    rest = a48.shape[2:]
    return (a48.reshape(16, 3, 8, 24, *rest)
            .transpose(0, 2, 1, 3, *range(4, 4 + len(rest)))
            .reshape(128, 72, *rest))


def from_pix(apix):
    """[128,72,...] -> [48,192,...]."""
    rest = apix.shape[2:]
    return (apix.reshape(16, 8, 3, 24, *rest)
            .transpose(0, 2, 1, 3, *range(4, 4 + len(rest)))
            .reshape(48, 192, *rest))


def pack_weights(ins, ns):
    """Block-diagonal lhsT packing of the two branch denoisers."""
    Wm_in, Wp_in = ins["Wm_in"], ins["Wp_in"]
    Wm_mid, Wp_mid = ins["Wm_mid"], ins["Wp_mid"]
    Wm_out, Wp_out = ins["Wm_out"], ins["Wp_out"]
    w18 = np.zeros((ns, 18, 128), np.float32)
    wmid = np.zeros((ns, 128, NMID, 9, 128), np.float32)
    wout = np.zeros((ns, 128, 9, 2), np.float32)
    for i in range(ns):
        for ky in range(3):
            for kx in range(3):
                t = ky * 3 + kx
                row = 0 if t == 4 else 2 + 2 * (t if t < 4 else t - 1)
                w18[i, row + 0, 0:64] = Wm_in[i, ky, kx, 0]
                w18[i, row + 1, 64:128] = Wp_in[i, ky, kx, 0]
                for l in range(NMID):
                    wmid[i, 0:64, l, t, 0:64] = Wm_mid[i, l, ky, kx]
                    wmid[i, 64:128, l, t, 64:128] = Wp_mid[i, l, ky, kx]
                wout[i, 0:64, t, 0] = Wm_out[i, ky, kx, :, 0]
                wout[i, 64:128, t, 1] = Wp_out[i, ky, kx, :, 0]
    bin_ = np.concatenate([ins["bm_in"][:ns], ins["bp_in"][:ns]], axis=1)  # [ns,128]
    bmid = np.concatenate([ins["bm_mid"][:ns], ins["bp_mid"][:ns]], axis=2)  # [ns,3,128]
    bmid = np.ascontiguousarray(bmid.transpose(0, 2, 1))  # [ns,128,3]
    bout = np.stack([np.repeat(ins["bm_out"][:ns, 0], 1),
                     np.repeat(ins["bp_out"][:ns, 0], 1)], axis=1)  # [ns,2]
    return w18, wmid, wout, bin_.astype(np.float32), bmid.astype(np.float32), \
        bout.astype(np.float32)


# ------------------------------ device program ------------------------------

def emit_jr(nc, pools, mode, xpix, bpix, tneg, scr, gs=0, ng=NG):
    """Emit one Jr evaluation on pixel groups [gs, gs+ng) of the [128, 720]
    tiles. Emitting two half-width calls per iteration lets the
    GPSIMD/ACT/DVE stages of the two halves pipeline instead of running the
    full-width chain serially.

    mode='loop': also applies the x <- rc(x - 2*jr) update in place.
    mode='eval': leaves E*g_m0 in scr.red[:,0,gs:] and E*g_p2 in scr.gep.
    """
    arg_f, sr_f, uz_f, red_f, gep_f = scr
    arg = arg_f[:, gs * E:(gs + ng) * E]
    sr = sr_f[:, gs * E:(gs + ng) * E]
    uz = uz_f[:, :, gs * E:(gs + ng) * E]
    red = red_f[:, :, gs:gs + ng]
    gep = gep_f[:, gs:gs + ng]
    bpix = bpix[:, gs * E:(gs + ng) * E]
    xm = xpix[:, gs:gs + ng]
    xp = xpix[:, NG + gs:NG + gs + ng]
    xm_b = xm[:, :, None].to_broadcast([128, ng, E])
    xp_b = xp[:, :, None].to_broadcast([128, ng, E])
    tneg_b = tneg[:, None, :].to_broadcast([128, ng, E])
    arg3 = arg.rearrange("p (g k) -> p g k", k=E)
    sr3 = sr.rearrange("p (g k) -> p g k", k=E)
    # arg = -t * p2
    nc.gpsimd.tensor_tensor(out=arg3, in0=tneg_b, in1=xp_b, op=ALU.mult)
    # e = exp(arg)  (in place)
    nc.scalar.activation(out=arg, in_=arg, func=ACTF.Exp)
    # s = e * m0
    nc.gpsimd.tensor_tensor(out=sr3, in0=arg3, in1=xm_b, op=ALU.mult)
    # r = s - b  (in place)
    nc.vector.tensor_tensor(out=sr, in0=sr, in1=bpix, op=ALU.subtract)
    # u = e * r
    nc.vector.tensor_tensor(out=uz[:, 0], in0=arg, in1=sr, op=ALU.mult)
    # z = u * (-t)
    nc.gpsimd.tensor_tensor(out=uz.rearrange("p c (g k) -> p c g k", k=E)[:, 1],
                            in0=uz.rearrange("p c (g k) -> p c g k", k=E)[:, 0],
                            in1=tneg_b, op=ALU.mult)
    # red[:,0]=sum_k u (=E*g0) ; red[:,1]=sum_k z (=E*g1/m0)
    nc.vector.tensor_reduce(out=red, in_=uz.rearrange("p c (g k) -> p c g k", k=E),
                            axis=mybir.AxisListType.X, op=ALU.add)
    # gep = m0 * red1  (=E*g1)
    nc.gpsimd.tensor_tensor(out=gep, in0=xm, in1=red[:, 1], op=ALU.mult)
    if mode == "loop":
        # m0' = clamp(m0 - (2/E) * red0)
        nc.vector.scalar_tensor_tensor(out=xm, in0=red[:, 0], scalar=-2.0 * EINV,
                                       in1=xm, op0=ALU.mult, op1=ALU.add)
        nc.vector.tensor_scalar(out=xm, in0=xm, scalar1=0.0, scalar2=M0_MAX,
                                op0=ALU.max, op1=ALU.min)
        nc.vector.scalar_tensor_tensor(out=xp, in0=gep, scalar=-2.0 * EINV,
                                       in1=xp, op0=ALU.mult, op1=ALU.add)
        nc.vector.tensor_scalar(out=xp, in0=xp, scalar1=0.0, scalar2=P2_MAX,
                                op0=ALU.max, op1=ALU.min)


def build_nc(niter, ns):
    nc = bacc.Bacc("TRN2", target_bir_lowering=False, debug=False,
                   num_devices=N_CORES)
    # ---- I/O ----
    bpix_d = nc.declare_dram_parameter("bpix", [128, NG * E], F32, isOutput=False)
    tneg_d = nc.declare_dram_parameter("tneg", [128, E], F32, isOutput=False)
    w18_d = nc.declare_dram_parameter("w18", [ns, 18, 128], F32, isOutput=False)
    wmid_d = nc.declare_dram_parameter("wmid", [ns, 128, NMID, 9, 128], F32,
                                       isOutput=False)
    wout_d = nc.declare_dram_parameter("wout", [ns, 128, 9 * 2], F32, isOutput=False)
    bin_d = nc.declare_dram_parameter("bin", [ns, 128], F32, isOutput=False)
    bmid_d = nc.declare_dram_parameter("bmid", [ns, 128, NMID], F32, isOutput=False)
    bout_d = nc.declare_dram_parameter("bout", [ns, 2], F32, isOutput=False)
    mask_d = nc.declare_dram_parameter("masks", [128, 2, HALO * RW], F32, isOutput=False)
    sc_d = nc.declare_dram_parameter("scal", [ns, 128, 2], F32, isOutput=False)  # [-mu, lm]
    nbr_d = nc.declare_dram_parameter("nbr", [1, 2], mybir.dt.uint32, isOutput=False)
    out_d = nc.declare_dram_parameter("out", [2 * ns + 1, 128, 2 * NG], F32,
                                      isOutput=True)
    # collective bounce buffers
    cc_in = nc.dram_tensor("cc_in", [2, 2 * HALO, RW], F32)
    cc_out = nc.dram_tensor("cc_out", [SLABS, 2, 2 * HALO, RW], F32)

    with tile.TileContext(nc) as tc:
        with (tc.tile_pool(name="big", bufs=1) as big,
              tc.tile_pool(name="wpool", bufs=2) as wpool,
              tc.tile_pool(name="scratch", bufs=1) as scratch,
              tc.tile_pool(name="psum", bufs=8, space="PSUM") as psum):
            # ---- persistent tiles ----
            bpix = big.tile([128, NG * E], F32)
            tneg = big.tile([128, E], F32)
            xpix = big.tile([128, 2 * NG], F32)
            x18 = big.tile([18, XBUF], F32)
            hA = big.tile([128, XBUF], F32)
            hB = big.tile([128, XBUF], F32)
            masks = big.tile([128, 2, HALO * RW], F32)
            nbr = big.tile([1, 2], mybir.dt.uint32)
            xmid = big.tile([16, 2, 3, 192], F32)
            arg = scratch.tile([128, NG * E], F32)
            sr = scratch.tile([128, NG * E], F32)
            uz = scratch.tile([128, 2, NG * E], F32)
            red = scratch.tile([128, 2, NG], F32)
            gep = scratch.tile([128, NG], F32)
            scr = (arg, sr, uz, red, gep)
            q = scratch.tile([128, 2 * NG], F32)
            dxc = scratch.tile([128, 2 * NG], F32)
            cres = scratch.tile([128, 2 * NG], F32)

            nc.sync.dma_start(out=bpix[:], in_=bpix_d[:])
            nc.sync.dma_start(out=tneg[:], in_=tneg_d[:])
            nc.sync.dma_start(out=masks[:], in_=mask_d[:])
            nc.sync.dma_start(out=nbr[:], in_=nbr_d[:])
            nc.vector.memset(x18[:], 0.0)
            nc.vector.memset(hA[:], 0.0)
            nc.vector.memset(hB[:], 0.0)

            # ---- init x ----
            bp3 = bpix.rearrange("p (g k) -> p g k", k=E)
            nc.vector.tensor_reduce(out=xpix[:, 0:NG], in_=bp3,
                                    axis=mybir.AxisListType.X, op=ALU.max)
            nc.vector.memset(xpix[:, NG:2 * NG], 1.0)

            # ---- Jr loop ----
            UNROLL = next(u for u in (10, 6, 5, 4, 3, 2, 1) if niter % u == 0)
            with tc.For_i(0, niter // UNROLL, 1):
                for _ in range(UNROLL):
                    emit_jr(nc, None, "loop", xpix, bpix, tneg, scr, 0, NG // 2)
                    emit_jr(nc, None, "loop", xpix, bpix, tneg, scr,
                            NG // 2, NG // 2)

            XIMG = 0  # x18 rows 0,1 double as the center-tap image buffer

            def _img_side_ap(t18, row0, ch, rm):
                # one partition row; dims (a:16, d*24+cm merged:192)
                return bass.AP(
                    tensor=t18.tensor,
                    offset=t18.offset + (row0 + ch) * t18.tensor.shape[1]
                    + GUARD + (HALO + 1 + rm) * RW + 1,
                    ap=[[t18.tensor.shape[1], 1], [3 * RW, 16], [1, 192]],
                )

            def reshape_pix_to_img(src_pix, dst18, row0):
                # own pixel (r,c): r=(a,rm), c=(d,cm); img f=(6+3a+rm)*194+1+24d+cm
                # two hops through xmid: [128,(rm cm)] -> [16=a, 192=(d cm)]
                for ch in range(2):
                    for rm in range(3):
                        src = src_pix[:, ch * NG + rm * 24: ch * NG + rm * 24 + 24]
                        nc.sync.dma_start(out=xmid[:, ch, rm, :], in_=src)
                for ch in range(2):
                    for rm in range(3):
                        nc.sync.dma_start(out=_img_side_ap(dst18, row0, ch, rm),
                                          in_=xmid[:, ch, rm, :])

            def reshape_img_to_pix(src18, dst_pix, row0):
                for ch in range(2):
                    for rm in range(3):
                        nc.sync.dma_start(out=xmid[:, ch, rm, :],
                                          in_=_img_side_ap(src18, row0, ch, rm))
                for ch in range(2):
                    for rm in range(3):
                        dst = dst_pix[:, ch * NG + rm * 24: ch * NG + rm * 24 + 24]
                        nc.sync.dma_start(out=dst, in_=xmid[:, ch, rm, :])

            # registers for neighbor ranks (prev, next within 4-core group)
            prev_rr = nc.sync.alloc_register("prev_r")
            next_rr = nc.sync.alloc_register("next_r")
            nc.sync.reg_load(prev_rr, nbr[0:1, 0:1])
            nc.sync.reg_load(next_rr, nbr[0:1, 1:2])
            prev_r = nc.sync.snap(prev_rr, donate=True, min_val=0,
                                  max_val=SLABS - 1)
            next_r = nc.sync.snap(next_rr, donate=True, min_val=0,
                                  max_val=SLABS - 1)

            def halo_exchange(stage):
                # own boundary rows -> cc_in
                src_top = bass.AP(
                    tensor=x18.tensor,
                    offset=x18.offset + XIMG * x18.tensor.shape[1] + GUARD
                    + (HALO + 1) * RW,
                    ap=[[x18.tensor.shape[1], 2], [1, HALO * RW]])
                src_bot = bass.AP(
                    tensor=x18.tensor,
                    offset=x18.offset + XIMG * x18.tensor.shape[1] + GUARD
                    + (1 + SH) * RW,
                    ap=[[x18.tensor.shape[1], 2], [1, HALO * RW]])
                nc.sync.dma_start(out=cc_in[:, 0:HALO, :], in_=src_top)
                nc.sync.dma_start(out=cc_in[:, HALO:2 * HALO, :], in_=src_bot)
                nc.gpsimd.collective_compute(
                    "AllGather", ALU.bypass,
                    replica_groups=[[0, 1, 2, 3], [4, 5, 6, 7]],
                    ins=[cc_in[:]], outs=[cc_out[:]],
                )
                # neighbor rows -> own halo rows of the image buffer
                dst_top = bass.AP(
                    tensor=x18.tensor,
                    offset=x18.offset + XIMG * x18.tensor.shape[1] + GUARD + RW,
                    ap=[[x18.tensor.shape[1], 2], [1, HALO * RW]])
                dst_bot = bass.AP(
                    tensor=x18.tensor,
                    offset=x18.offset + XIMG * x18.tensor.shape[1] + GUARD
                    + (1 + HALO + SH) * RW,
                    ap=[[x18.tensor.shape[1], 2], [1, HALO * RW]])
                nc.sync.dma_start(
                    out=dst_top, in_=cc_out[bass.ds(prev_r, 1), :, HALO:2 * HALO, :])
                nc.sync.dma_start(
                    out=dst_bot, in_=cc_out[bass.ds(next_r, 1), :, 0:HALO, :])
                # mask the halo strips (zero at image boundary, 1 interior)
                st = bass.AP(tensor=x18.tensor,
                             offset=x18.offset + XIMG * x18.tensor.shape[1]
                             + GUARD + RW,
                             ap=[[x18.tensor.shape[1], 2], [1, HALO * RW]])
                sb = bass.AP(tensor=x18.tensor,
                             offset=x18.offset + XIMG * x18.tensor.shape[1]
                             + GUARD + (1 + HALO + SH) * RW,
                             ap=[[x18.tensor.shape[1], 2], [1, HALO * RW]])
                nc.vector.tensor_tensor(out=st, in0=st, in1=masks[0:2, 0],
                                        op=ALU.mult)
                nc.vector.tensor_tensor(out=sb, in0=sb, in1=masks[0:2, 1],
                                        op=ALU.mult)

            def build_taps():
                # copy center rows into the other 8 tap rows, shifted
                for ky in range(3):
                    for kx in range(3):
                        t = ky * 3 + kx
                        if t == 4:
                            continue
                        row = 2 + 2 * (t if t < 4 else t - 1)
                        d = (ky - 1) * RW + (kx - 1)
                        src = bass.AP(
                            tensor=x18.tensor,
                            offset=x18.offset + XIMG * x18.tensor.shape[1]
                            + GUARD + d,
                            ap=[[x18.tensor.shape[1], 2], [1, SPAN]])
                        dst = bass.AP(
                            tensor=x18.tensor,
                            offset=x18.offset + row * x18.tensor.shape[1] + GUARD,
                            ap=[[x18.tensor.shape[1], 2], [1, SPAN]])
                        nc.sync.dma_start(out=dst, in_=src)

            def zero_hpads(h):
                # pad cols 0 and 193 of rows 1..58
                ap = bass.AP(tensor=h.tensor, offset=h.offset + GUARD + RW,
                             ap=[[h.tensor.shape[1], 128], [RW, NROW - 2],
                                 [RW - 1, 2]])
                nc.vector.memset(ap, 0.0)

            def mask_h(h):
                for (base, m) in ((RW, 0), ((1 + HALO + SH) * RW, 1)):
                    ap = bass.AP(tensor=h.tensor, offset=h.offset + GUARD + base,
                                 ap=[[h.tensor.shape[1], 128], [1, HALO * RW]])
                    nc.vector.tensor_tensor(
                        out=ap, in0=ap, in1=masks[:, m], op=ALU.mult)

            def conv_layer(rhs_tile, rhs_nrow, lhsT, ntap, bias_ap, relu, out_h,
                           out_np, evict_alt):
                """One conv layer: ntap matmuls per chunk accumulating in PSUM,
                then bias(+relu) eviction into out_h rows [0:out_np]."""
                for c in range(NCHUNK):
                    base = CHUNK0 + 512 * c
                    n = min(512, CHUNK0 + NPIX_F - base)
                    ps = psum.tile([128, 512], F32, tag="ps")
                    for t in range(ntap):
                        if ntap == 1:
                            rhs = bass.AP(
                                tensor=rhs_tile.tensor,
                                offset=rhs_tile.offset + GUARD + base,
                                ap=[[rhs_tile.tensor.shape[1], rhs_nrow], [1, n]])
                            nc.tensor.matmul(ps[:out_np, :n], lhsT, rhs,
                                             start=True, stop=True)
                        else:
                            d = ((t // 3) - 1) * RW + (t % 3) - 1
                            rhs = bass.AP(
                                tensor=rhs_tile.tensor,
                                offset=rhs_tile.offset + GUARD + base + d,
                                ap=[[rhs_tile.tensor.shape[1], rhs_nrow], [1, n]])
                            nc.tensor.matmul(ps[:out_np, :n], lhsT[t], rhs,
                                             start=(t == 0), stop=(t == ntap - 1))
                    dst = bass.AP(tensor=out_h.tensor,
                                  offset=out_h.offset + GUARD + base,
                                  ap=[[out_h.tensor.shape[1], out_np], [1, n]])
                    eng = nc.vector if (c % 2 == 0 or not evict_alt) else nc.scalar
                    if relu:
                        if eng is nc.vector:
                            nc.vector.tensor_scalar(
                                out=dst, in0=ps[:out_np, :n], scalar1=bias_ap,
                                scalar2=0.0, op0=ALU.add, op1=ALU.max)
                        else:
                            nc.scalar.activation(out=dst, in_=ps[:out_np, :n],
                                                 func=ACTF.Relu, bias=bias_ap)
                    else:
                        nc.vector.tensor_scalar(
                            out=dst, in0=ps[:out_np, :n], scalar1=bias_ap,
                            scalar2=0.0, op0=ALU.add, op1=ALU.bypass)

            # ---------------- stages ----------------
            for i in range(ns):
                w18s = wpool.tile([18, 128], F32, tag="w18")
                wmids = wpool.tile([128, NMID, 9, 128], F32, tag="wmid")
                wouts = wpool.tile([128, 9 * 2], F32, tag="wout")
                bins = wpool.tile([128, 1], F32, tag="bin")
                bmids = wpool.tile([128, NMID], F32, tag="bmid")
                bouts = wpool.tile([2, 1], F32, tag="bout")
                scs = wpool.tile([128, 2], F32, tag="scs")
                nc.sync.dma_start(out=w18s[:], in_=w18_d[i])
                nc.sync.dma_start(out=wmids[:], in_=wmid_d[i])
                nc.sync.dma_start(out=wouts[:], in_=wout_d[i])
                nc.sync.dma_start(out=bins[:], in_=bin_d[i, :, None])
                nc.sync.dma_start(out=bmids[:], in_=bmid_d[i])
                nc.sync.dma_start(out=bouts[:], in_=bout_d[i, :, None])
                nc.sync.dma_start(out=scs[:], in_=sc_d[i])

                reshape_pix_to_img(xpix, x18, XIMG)
                ximg_pads = bass.AP(
                    tensor=x18.tensor,
                    offset=x18.offset + XIMG * x18.tensor.shape[1] + GUARD + RW,
                    ap=[[x18.tensor.shape[1], 2], [RW, NROW - 2], [RW - 1, 2]])
                nc.vector.memset(ximg_pads, 0.0)
                halo_exchange(i)
                build_taps()
                # conv1: K=18 single matmul per chunk
                conv_layer(x18, 18, w18s[:], 1, bins[:], True, hA, 128, True)
                zero_hpads(hA); mask_h(hA)
                hs = [hA, hB]
                for l in range(NMID):
                    hin, hout = hs[l % 2], hs[(l + 1) % 2]
                    conv_layer(hin, 128,
                               [wmids[:, l, t, :] for t in range(9)], 9,
                               bmids[:, l:l + 1], True, hout, 128, True)
                    zero_hpads(hout); mask_h(hout)
                hlast = hs[NMID % 2]
                # conv5 -> write into x18 rows XIMG (overwrite image; it is
                # no longer needed) -- use hA/hB other buffer? use dedicated:
                conv_layer(hlast, 128,
                           [wouts[:, 2 * t:2 * t + 2] for t in range(9)], 9,
                           bouts[:], False, x18, 2, False)
                # conv result (rows 0,1 of x18) -> pixel layout
                reshape_img_to_pix(x18, cres, 0)
                # Dx = rc(x - cres)
                nc.vector.tensor_tensor(out=dxc, in0=xpix, in1=cres, op=ALU.subtract)
                nc.vector.tensor_scalar(out=dxc[:, 0:NG], in0=dxc[:, 0:NG],
                                        scalar1=0.0, scalar2=M0_MAX,
                                        op0=ALU.max, op1=ALU.min)
                nc.vector.tensor_scalar(out=dxc[:, NG:2 * NG], in0=dxc[:, NG:2 * NG],
                                        scalar1=0.0, scalar2=P2_MAX,
                                        op0=ALU.max, op1=ALU.min)
                nc.sync.dma_start(out=out_d[2 * i], in_=dxc[:])
                # jr eval
                emit_jr(nc, None, "eval", xpix, bpix, tneg, scr)
                # q = x - Dx ; w1 = q*lm ; upd = gE/E + w1 ; x' = rc(x - mu*upd)
                nc.vector.tensor_tensor(out=q, in0=xpix, in1=dxc, op=ALU.subtract)
                nc.vector.tensor_scalar(out=q, in0=q, scalar1=scs[:, 1:2],
                                        scalar2=1.0, op0=ALU.mult, op1=ALU.mult)
                nc.vector.scalar_tensor_tensor(out=q[:, 0:NG], in0=red[:, 0],
                                               scalar=EINV, in1=q[:, 0:NG],
                                               op0=ALU.mult, op1=ALU.add)
                nc.vector.scalar_tensor_tensor(out=q[:, NG:2 * NG], in0=gep,
                                               scalar=EINV, in1=q[:, NG:2 * NG],
                                               op0=ALU.mult, op1=ALU.add)
                # x' = x + negmu*q
                nc.vector.tensor_scalar(out=q, in0=q, scalar1=scs[:, 0:1],
                                        scalar2=1.0, op0=ALU.mult, op1=ALU.mult)
                nc.vector.tensor_tensor(out=xpix, in0=xpix, in1=q, op=ALU.add)
                nc.vector.tensor_scalar(out=xpix[:, 0:NG], in0=xpix[:, 0:NG],
                                        scalar1=0.0, scalar2=M0_MAX,
                                        op0=ALU.max, op1=ALU.min)
                nc.vector.tensor_scalar(out=xpix[:, NG:2 * NG],
                                        in0=xpix[:, NG:2 * NG],
                                        scalar1=0.0, scalar2=P2_MAX,
                                        op0=ALU.max, op1=ALU.min)
                nc.sync.dma_start(out=out_d[2 * i + 1], in_=xpix[:])
            nc.sync.dma_start(out=out_d[2 * ns], in_=xpix[:])
    nc.compile()
    return nc


# ------------------------------- host wrapper -------------------------------

def _prep_inputs(ins, niter, ns):
    b = ins["b"].astype(np.float32)
    tes = ins["tes"].astype(np.float32)
    w18, wmid, wout, bin_, bmid, bout = pack_weights(ins, ns)
    sc = np.stack([-ins["mu"][:ns], ins["lm"][:ns]], axis=1).astype(np.float32)
    sc = np.ascontiguousarray(np.tile(sc[:, None, :], (1, 128, 1)))
    in_maps = []
    for core in range(N_CORES):
        bi, s = divmod(core, SLABS)
        bslab = b[bi, s * SH:(s + 1) * SH]          # [48,192,10]
        bpix = to_pix(bslab).reshape(128, NG * E)
        tneg = np.tile(-tes[bi][None, :], (128, 1)).astype(np.float32)
        masks = np.ones((2, HALO * RW), np.float32)
        if s == 0:
            masks[0] = 0.0
        if s == SLABS - 1:
            masks[1] = 0.0
        masks = np.tile(masks[None], (128, 1, 1))
        nbr = np.array([[max(0, s - 1), min(SLABS - 1, s + 1)]], np.uint32)
        in_maps.append({
            "bpix": np.ascontiguousarray(bpix),
            "tneg": tneg,
            "w18": w18[:ns], "wmid": wmid[:ns],
            "wout": wout[:ns].reshape(ns, 128, 18),
            "bin": bin_[:ns], "bmid": bmid[:ns], "bout": bout[:ns],
            "masks": masks, "scal": sc, "nbr": nbr,
        })
    return in_maps


def _run(ins, niter, ns, trace=False):
    key = (niter, ns)
    if key not in _CACHE:
        _CACHE[key] = build_nc(niter, ns)
    nc = _CACHE[key]
    in_maps = _prep_inputs(ins, niter, ns)
    r = run_bass_kernel_spmd(nc, in_maps, core_ids=list(range(N_CORES)),
                             trace=trace)
    nout = 2 * ns + 1
    full = np.zeros((nout, B, H, W, 2), np.float32)
    for core in range(N_CORES):
        bi, s = divmod(core, SLABS)
        o = r.results[core]["out"].reshape(nout, 128, 2, NG)
        o = np.moveaxis(o, 2, 3)                      # [n,128,NG,2]
        full[:, bi, s * SH:(s + 1) * SH] = np.stack(
            [from_pix(o[j]) for j in range(nout)])
    return full, r


def kernel(**inputs):
    ins = {k: np.asarray(v) for k, v in inputs.items()}
    full, _ = _run(ins, NITER, NS)
    return full
